# revision 14
# baseline (speedup 1.0000x reference)
"""BinaryNormalizedConv2d on 8 Trainium2 NeuronCores.

Reference computation (per full input):
  Wq = (w > mean(w)), bq = (b > mean(b))          # {0,1} f32
  z  = conv2d(x, Wq, stride 1, pad 1) + bq
  z  = (z - mean_b(z)) / (sqrt(var_b(z, ddof=1)) + 1e-5)   # per-sample over (C,H,W)
  out = relu(z)

Sharding: data-parallel over batch (32 -> 4 per core), weights replicated.

Device kernel (per core, B=4, Cin=128, Cout=256, H=W=56):
  - Mixed-precision conv: the 4 corner taps (kh,kw) in {0,2}x{0,2} run as 2
    fp8e4m3 DoubleRow matmuls (2 taps per instruction, ~2 fp8 elem/cycle),
    the remaining 5 taps as bf16 matmuls. Weights {0,1} are exact in both
    dtypes; only x quantization contributes error (measured 1.68e-2
    absmax-relative vs the 2e-2 gate).
  - x stored padded per-sample [Cin=128 partitions, b, 58*58+4] in bf16 AND
    fp8 so each tap (kh,kw) is a flat offset kh*58+kw. DoubleRow rhs is a
    [128, 2, 464] AP whose dim-1 stride pairs two shifted overlapping views.
  - PSUM banks are 464 wide (8 rows x 58); DR taps write all 464 columns
    (16 garbage), bf16 taps and the evac use tight 8x56 strided views.
  - PSUM evac via VectorE tensor_scalar copy with accum_out row sums; sum of
    squares via ScalarE Square (GpSimd for the last sample to keep ACT/DVE
    free at the tail).
  - Per-sample stats: one fused grouped tensor_reduce + ones-matmul
    partition-broadcast; inv-std via ScalarE Rsqrt (eps dropped: effect
    ~4e-7 relative). The stats matmul of sample b is emitted after sample
    b+1's first conv group so the PE never stalls on the DVE reduce.
  - normalize+relu to fp16: samples 0-2 on ScalarE (hidden under conv);
    the last sample is split ScalarE/VectorE/GpSimd with output DMAs on
    three queues so the serial tail is ~7us instead of ~16us.
  - Output is fp16 (absmax ~5.4, adds <1e-4 error); host upcasts to f32.
"""

import numpy as np
import ml_dtypes
from contextlib import ExitStack

# ---- problem constants (hardcoded per contract) ----
B_FULL, CIN, H, W = 32, 128, 56, 56
COUT, KK = 256, 3
N_CORES = 8
B = B_FULL // N_CORES          # 4 samples per core
HP = H + 2                     # 58 padded rows/cols
SB_B = HP * HP + 4             # per-sample stride in padded x (3368)
YB = 7                         # y-blocks
RPB = H // YB                  # 8 rows per block
NFREE = RPB * HP               # 464 matmul free size (incl 2 garbage cols/row)
NINT = RPB * W                 # 448 interior elements per block
HW = H * W                     # 3136
NELEM = COUT * HW              # 802816 elements per sample for stats
EPS = 1e-5

# conv taps: corners in fp8 DoubleRow pairs, the rest bf16
FP8_PAIRS = [((0, 0), (0, 2)), ((2, 0), (2, 2))]
BF_TAPS = [(0, 1), (1, 0), (1, 1), (1, 2), (2, 1)]

_CACHE = {}
TRACE = False                  # set by test.py to collect an NTFF profile
TRACE_DIR = None
LAST_RESULTS = None


def _off(t):
    return t[0] * HP + t[1]


def _emit(nc, tc, xb_d, x8_d, wb_d, w8_d, bq_d, y_d):
    import concourse.mybir as mybir
    from concourse.ap import AP

    f32 = mybir.dt.float32
    f16 = mybir.dt.float16
    bf16 = mybir.dt.bfloat16
    fp8 = mybir.dt.float8e4
    AF = mybir.ActivationFunctionType
    OP = mybir.AluOpType
    AX = mybir.AxisListType
    DR = mybir.MatmulPerfMode.DoubleRow

    NBF = len(BF_TAPS)

    with ExitStack() as ctx:
        const_pool = ctx.enter_context(tc.tile_pool(name="const", bufs=1))
        xpool = ctx.enter_context(tc.tile_pool(name="x", bufs=1))
        zpool = ctx.enter_context(tc.tile_pool(name="z", bufs=5))
        sqpool = ctx.enter_context(tc.tile_pool(name="sq", bufs=3))
        stpool = ctx.enter_context(tc.tile_pool(name="st", bufs=2))
        outpool = ctx.enter_context(tc.tile_pool(name="out", bufs=8))
        cpsum = ctx.enter_context(tc.tile_pool(name="cps", bufs=7, space="PSUM"))
        spsum = ctx.enter_context(tc.tile_pool(name="sps", bufs=1, space="PSUM"))

        # PE warm-up first in program order: scr memset on GpSimd issues
        # before any DMA work, so the HAM ramp starts while inputs land.
        scr = const_pool.tile([128, 576], bf16)
        nc.gpsimd.memset(scr[:], 0.0)
        for _ in range(8):
            dzt = cpsum.tile([128, NFREE], f32, tag="zt")
            nc.tensor.matmul(dzt[:, 0:448], scr[:, 0:128], scr[:, 0:448],
                             start=True, stop=True)

        wb_sb = const_pool.tile([CIN, 2 * NBF * 128], bf16)
        WHALF = NBF * 128
        nc.sync.dma_start(wb_sb[:, 0:WHALF], wb_d[:, 0:WHALF])
        nc.scalar.dma_start(wb_sb[:, WHALF:], wb_d[:, WHALF:])
        w8_sb = const_pool.tile([CIN, 2 * 2 * 256], fp8)
        nc.gpsimd.dma_start(w8_sb[:], w8_d[:])
        bq_sb = const_pool.tile([128, 5], f32)
        nc.sync.dma_start(bq_sb[:], bq_d[:])
        ones = const_pool.tile([128, 128], f32)
        nc.vector.memset(ones[:], 1.0)

        # x: bf16 copy on the scalar queue, fp8 copy on the vector queue.
        xb_sb = xpool.tile([CIN, B * SB_B], bf16)
        x8_sb = xpool.tile([CIN, B * SB_B], fp8)
        cuts = [0, NFREE + 3 * HP, 3 * NFREE + 2 * HP, 5 * NFREE + 2 * HP, SB_B]
        nc.scalar.dma_start(xb_sb[:, 0:cuts[1]], xb_d[:, 0:cuts[1]])
        nc.gpsimd.dma_start(x8_sb[:, 0:cuts[1]], x8_d[:, 0:cuts[1]])
        for i in range(1, 4):
            nc.scalar.dma_start(xb_sb[:, cuts[i]:cuts[i + 1]],
                                xb_d[:, cuts[i]:cuts[i + 1]])
            nc.gpsimd.dma_start(x8_sb[:, cuts[i]:cuts[i + 1]],
                                x8_d[:, cuts[i]:cuts[i + 1]])
        for b in range(1, B):
            s = slice(b * SB_B, (b + 1) * SB_B)
            nc.scalar.dma_start(xb_sb[:, s], xb_d[:, s])
            nc.gpsimd.dma_start(x8_sb[:, s], x8_d[:, s])

        x8_base = x8_sb[:]
        p_stride = list(x8_base.ap[0])

        def dr_rhs(o0, tA, tB):
            return AP(x8_base.tensor, x8_base.offset + o0 + _off(tA),
                      [p_stride, [_off(tB) - _off(tA), 2], [1, NFREE]])

        def emit_partial_reduce(rsq, part):
            # partial [S0,S1,Q0,Q1] over yb0..4 while yb5,6 still compute
            r = rsq[:]
            rin = AP(r.tensor, r.offset, [list(r.ap[0]), [8, 4], [1, 5]])
            nc.vector.tensor_reduce(part[:], rin, axis=AX.X, op=OP.add)

        def emit_stats_and_normalize(b, rsq, z_hb, part, last):
            # ---- per-sample stats ----
            st6 = stpool.tile([128, 6], f32, tag="st6")
            r = rsq[:]
            rt = AP(r.tensor, r.offset + 5, [list(r.ap[0]), [8, 4], [1, 2]])
            t2 = stpool.tile([128, 4], f32, tag="t2")
            nc.vector.tensor_reduce(t2[:], rt, axis=AX.X, op=OP.add)
            nc.vector.tensor_tensor(st6[:, 0:4], part[:], t2[:], op=OP.add)
            # BR2 = S_c * 2*bq_c
            nc.vector.tensor_tensor(st6[:, 4:6], st6[:, 0:2], bq_sb[:, 3:5],
                                    op=OP.mult)
            st_ps = spsum.tile([128, 6], f32, tag="stps")
            nc.tensor.matmul(st_ps[:], ones[:], st6[:], start=True, stop=True)
            sb_st = stpool.tile([128, 6], f32, tag="sbst")
            nc.vector.tensor_copy(sb_st[:], st_ps[:])

            # scal cols: 0 Stot, 1 mean, 2 SStot, 3 varN, 4 inv, 5 tmp
            scal = stpool.tile([128, 6], f32, tag="scal")
            # pairwise [S,Q,BR] = st[0::2] + st[1::2] in one strided tt
            pair = stpool.tile([128, 3], f32, tag="pair")
            sb = sb_st[:]
            in0 = AP(sb.tensor, sb.offset, [list(sb.ap[0]), [2, 3], [1, 1]])
            in1 = AP(sb.tensor, sb.offset + 1, [list(sb.ap[0]), [2, 3], [1, 1]])
            nc.vector.tensor_tensor(pair[:].rearrange("p (a b) -> p a b", b=1),
                                    in0, in1, op=OP.add)
            # Stot = S + C1; mean = Stot/NELEM
            nc.vector.tensor_tensor(scal[:, 0:1], pair[:, 0:1], bq_sb[:, 2:3],
                                    op=OP.add)
            nc.vector.tensor_scalar_mul(scal[:, 1:2], scal[:, 0:1], 1.0 / NELEM)
            # SStot = Q + BR2 + C1
            nc.vector.tensor_tensor(scal[:, 2:3], pair[:, 1:2], pair[:, 2:3],
                                    op=OP.add)
            nc.vector.tensor_tensor(scal[:, 2:3], scal[:, 2:3], bq_sb[:, 2:3],
                                    op=OP.add)
            # varN = SStot - Stot^2/NELEM
            nc.vector.scalar_tensor_tensor(scal[:, 5:6], scal[:, 0:1],
                                           1.0 / NELEM, scal[:, 0:1],
                                           op0=OP.mult, op1=OP.mult)
            nc.vector.tensor_tensor(scal[:, 3:4], scal[:, 2:3], scal[:, 5:6],
                                    op=OP.subtract)
            # inv = 1/sqrt(varN/(NELEM-1))  (eps dropped, ~4e-7 effect)
            nc.scalar.activation(scal[:, 4:5], scal[:, 3:4], AF.Sqrt,
                                 scale=1.0 / (NELEM - 1))
            nc.vector.reciprocal(scal[:, 4:5], scal[:, 4:5])
            # b2[h] = (bq[h] - mean) * inv
            b2 = stpool.tile([128, 2], f32, tag="b2")
            for h in range(2):
                nc.vector.scalar_tensor_tensor(b2[:, h:h + 1], bq_sb[:, h:h + 1],
                                               scal[:, 1:2], scal[:, 4:5],
                                               op0=OP.subtract, op1=OP.mult)

            # ---- normalize + relu(fp16) + store ----
            inv = scal[:, 4:5]
            if not last:
                HChunk = HW // 4
                for h in range(2):
                    for ck in range(4):
                        zn = outpool.tile([128, HChunk], f16, tag="zn")
                        zsrc = z_hb[h][:, ck * HChunk:(ck + 1) * HChunk]
                        nc.scalar.activation(zn[:], zsrc, AF.Relu,
                                             bias=b2[:, h:h + 1], scale=inv)
                        nc.sync.dma_start(
                            y_d[b, h * 128:(h + 1) * 128,
                                ck * HChunk:(ck + 1) * HChunk], zn[:])
            else:
                # ascending chunk sizes: small chunks first so the output
                # stream starts as early as possible; DMAs alternate queues
                s_cuts = [0, 420, 1708]       # ScalarE chunks
                v_cuts = [1708, 2240, 3136]   # VectorE chunks
                for h in range(2):
                    zs = z_hb[h]
                    row = y_d[b, h * 128:(h + 1) * 128]
                    for ci in range(2):
                        a, e = s_cuts[ci], s_cuts[ci + 1]
                        zn_s = outpool.tile([128, e - a], f16, tag="zns", bufs=4)
                        nc.scalar.activation(zn_s[:], zs[:, a:e], AF.Relu,
                                             bias=b2[:, h:h + 1], scale=inv)
                        q = nc.sync if (h + ci) % 2 == 0 else nc.gpsimd
                        q.dma_start(row[:, a:e], zn_s[:])
                        a, e = v_cuts[ci], v_cuts[ci + 1]
                        t_v = outpool.tile([128, e - a], f32, tag="tv", bufs=4)
                        nc.vector.tensor_scalar(out=t_v[:], in0=zs[:, a:e],
                                                scalar1=inv,
                                                scalar2=b2[:, h:h + 1],
                                                op0=OP.mult, op1=OP.add)
                        zn_v = outpool.tile([128, e - a], f16, tag="znv", bufs=4)
                        nc.vector.tensor_scalar_max(zn_v[:], t_v[:], 0.0)
                        q = nc.gpsimd if (h + ci) % 2 == 0 else nc.sync
                        q.dma_start(row[:, a:e], zn_v[:])

        for b in range(B):
            z_hb = []
            rsq = stpool.tile([128, 32], f32, tag="rsq")
            part = stpool.tile([128, 4], f32, tag="part")
            for h in range(2):
                z_sb = zpool.tile([128, HW], f32, tag="z")
                z_hb.append(z_sb)
                for yb in range(YB):
                    zt = cpsum.tile([128, NFREE], f32, tag="zt")
                    zt3 = zt[:].rearrange("p (r c) -> p r c", c=HP)[:, :, 0:W]
                    o0 = b * SB_B + yb * NFREE
                    for pi, (tA, tB) in enumerate(FP8_PAIRS):
                        lhs = w8_sb[:, (h * 2 + pi) * 256:(h * 2 + pi + 1) * 256
                                    ].rearrange("p (two m) -> p two m", two=2)
                        nc.tensor.matmul(zt[:], lhs, dr_rhs(o0, tA, tB),
                                         start=(pi == 0), stop=False,
                                         perf_mode=DR)
                    for ti, t in enumerate(BF_TAPS):
                        rhs = xb_sb[:, o0 + _off(t): o0 + _off(t) + NFREE
                                    ].rearrange("p (r c) -> p r c",
                                                c=HP)[:, :, 0:W]
                        nc.tensor.matmul(
                            zt3, wb_sb[:, (h * NBF + ti) * 128:
                                       (h * NBF + ti + 1) * 128], rhs,
                            start=False, stop=(ti == NBF - 1),
                            skip_group_check=True)
                    # evac (strided valid view) + per-channel row sums
                    si = h * 8 + yb
                    nc.vector.tensor_scalar(
                        out=z_sb[:, yb * NINT:(yb + 1) * NINT], in0=zt3,
                        scalar1=1.0, scalar2=0.0,
                        op0=OP.mult, op1=OP.add,
                        accum_out=rsq[:, si:si + 1])
                    zslice = z_sb[:, yb * NINT:(yb + 1) * NINT]
                    sq = sqpool.tile([128, NINT], f32, tag="sq")
                    if h == 0 or (b == B - 1 and yb >= 5):
                        # split sumsq between DVE and ACT so neither engine
                        # saturates (ACT also runs the prev sample's Relus);
                        # the last blocks go on DVE so the stats reduce isn't
                        # gated by queued ACT Relus
                        nc.vector.scalar_tensor_tensor(
                            out=sq[:], in0=zslice, scalar=1.0, in1=zslice,
                            op0=OP.mult, op1=OP.mult,
                            accum_out=rsq[:, 16 + si:17 + si])
                    else:
                        nc.scalar.activation(
                            sq[:], zslice, AF.Square,
                            accum_out=rsq[:, 16 + si:17 + si])
                    if h == 1 and yb == 4:
                        emit_partial_reduce(rsq, part)
            emit_stats_and_normalize(b, rsq, z_hb, part, last=(b == B - 1))


def _build_program():
    import concourse.bacc as bacc
    import concourse.tile as tile
    import concourse.mybir as mybir

    f32 = mybir.dt.float32
    f16 = mybir.dt.float16
    bf16 = mybir.dt.bfloat16
    fp8 = mybir.dt.float8e4

    nc = bacc.Bacc("TRN2", target_bir_lowering=False, debug=False, num_devices=1)

    xb_d = nc.dram_tensor("xb", [CIN, B * SB_B], bf16, kind="ExternalInput").ap()
    x8_d = nc.dram_tensor("x8", [CIN, B * SB_B], fp8, kind="ExternalInput").ap()
    wb_d = nc.dram_tensor("wb", [CIN, 2 * len(BF_TAPS) * 128], bf16,
                          kind="ExternalInput").ap()
    w8_d = nc.dram_tensor("w8", [CIN, 2 * 2 * 256], fp8,
                          kind="ExternalInput").ap()
    bq_d = nc.dram_tensor("bq", [128, 5], f32, kind="ExternalInput").ap()
    y_d = nc.dram_tensor("y", [B, COUT, HW], f16, kind="ExternalOutput").ap()

    with tile.TileContext(nc) as tc:
        _emit(nc, tc, xb_d, x8_d, wb_d, w8_d, bq_d, y_d)

    nc.compile()
    return nc


def _get_program():
    if "nc" not in _CACHE:
        _CACHE["nc"] = _build_program()
    return _CACHE["nc"]


def _binarize(t_np):
    """(t > t.mean()) as f32, matching the reference's jnp computation."""
    try:
        import jax.numpy as jnp
        tj = jnp.asarray(t_np)
        return np.asarray((tj > tj.mean()).astype(jnp.float32))
    except Exception:
        return (t_np > np.float32(t_np.astype(np.float64).mean())).astype(np.float32)


def kernel(x, weight, bias, train_mode=None):
    """Full-input entry point: shards over 8 NeuronCores, returns full output."""
    import time
    last_err = None
    for attempt in range(3):
        try:
            return _kernel_impl(x, weight, bias)
        except Exception as e:  # transient NRT/device errors: back off and retry
            last_err = e
            if attempt < 2:
                time.sleep(20.0 * (attempt + 1))
    raise last_err


def _kernel_impl(x, weight, bias):
    global LAST_RESULTS
    from concourse.bass_utils import run_bass_kernel_spmd

    x = np.asarray(x, dtype=np.float32)
    weight = np.asarray(weight, dtype=np.float32)
    bias = np.asarray(bias, dtype=np.float32)

    wq = _binarize(weight)                       # [256,128,3,3] {0,1}
    bq = _binarize(bias)                         # [256] {0,1}

    w4 = wq.reshape(2, 128, CIN, 3, 3)           # [h, co_l, ci, kh, kw]

    # bf16 taps -> lhsT layout [ci, (h, t, co_l)]
    wb = np.zeros((CIN, 2, len(BF_TAPS), 128), np.float32)
    for ti, (kh, kw) in enumerate(BF_TAPS):
        wb[:, :, ti, :] = w4[:, :, :, kh, kw].transpose(2, 0, 1)
    wb = np.ascontiguousarray(wb.reshape(CIN, -1)).astype(ml_dtypes.bfloat16)

    # fp8 DR pairs -> lhsT layout [ci, (h, pair, two, co_l)]
    w8 = np.zeros((CIN, 2, 2, 2, 128), np.float32)
    for pi, (tA, tB) in enumerate(FP8_PAIRS):
        for two, (kh, kw) in enumerate((tA, tB)):
            w8[:, :, pi, two, :] = w4[:, :, :, kh, kw].transpose(2, 0, 1)
    w8 = np.ascontiguousarray(w8.reshape(CIN, -1)).astype(ml_dtypes.float8_e4m3)

    bq2 = np.zeros((128, 5), np.float32)
    bq2[:, 0] = bq[0:128]
    bq2[:, 1] = bq[128:256]
    bq2[:, 2] = HW * bq.sum()                    # C1 constant, replicated
    bq2[:, 3] = 2.0 * bq[0:128]
    bq2[:, 4] = 2.0 * bq[128:256]

    # x -> padded [b, ci, SB_B] in bf16 and fp8
    xall_b = np.zeros((B_FULL, CIN, SB_B), dtype=ml_dtypes.bfloat16)
    xall_8 = np.zeros((B_FULL, CIN, SB_B), dtype=ml_dtypes.float8_e4m3)
    xv = xall_b[:, :, :HP * HP].reshape(B_FULL, CIN, HP, HP)
    xv[:, :, 1:H + 1, 1:W + 1] = x.astype(ml_dtypes.bfloat16)
    xv8 = xall_8[:, :, :HP * HP].reshape(B_FULL, CIN, HP, HP)
    xv8[:, :, 1:H + 1, 1:W + 1] = x.astype(ml_dtypes.float8_e4m3)

    in_maps = []
    for c in range(N_CORES):
        xc = np.ascontiguousarray(
            xall_b[c * B:(c + 1) * B].transpose(1, 0, 2).reshape(CIN, B * SB_B))
        xc8 = np.ascontiguousarray(
            xall_8[c * B:(c + 1) * B].transpose(1, 0, 2).reshape(CIN, B * SB_B))
        in_maps.append({"xb": xc, "x8": xc8, "wb": wb, "w8": w8, "bq": bq2})

    nc = _get_program()
    kwargs = {}
    if TRACE:
        kwargs = dict(trace=True, tmpdir=TRACE_DIR)
    res = run_bass_kernel_spmd(nc, in_maps, core_ids=list(range(N_CORES)), **kwargs)
    LAST_RESULTS = res

    out = np.concatenate([np.asarray(res.results[c]["y"], dtype=np.float32)
                          for c in range(N_CORES)], axis=0)
    return out.reshape(B_FULL, COUT, H, W)


# revision 15
# speedup vs baseline: 1.0330x; 1.0330x over previous
"""BinaryNormalizedConv2d on 8 Trainium2 NeuronCores.

Reference computation (per full input):
  Wq = (w > mean(w)), bq = (b > mean(b))          # {0,1} f32
  z  = conv2d(x, Wq, stride 1, pad 1) + bq
  z  = (z - mean_b(z)) / (sqrt(var_b(z, ddof=1)) + 1e-5)   # per-sample over (C,H,W)
  out = relu(z)

Sharding: data-parallel over batch (32 -> 4 per core), weights replicated.

Device kernel (per core, B=4, Cin=128, Cout=256, H=W=56):
  - Mixed-precision conv: the 4 corner taps (kh,kw) in {0,2}x{0,2} run as 2
    fp8e4m3 DoubleRow matmuls (2 taps per instruction, ~2 fp8 elem/cycle),
    the remaining 5 taps as bf16 matmuls. Weights {0,1} are exact in both
    dtypes; only x quantization contributes error (measured 1.68e-2
    absmax-relative vs the 2e-2 gate).
  - x stored padded per-sample [Cin=128 partitions, b, 58*58+4] in bf16 AND
    fp8 so each tap (kh,kw) is a flat offset kh*58+kw. DoubleRow rhs is a
    [128, 2, 464] AP whose dim-1 stride pairs two shifted overlapping views.
  - PSUM banks are 464 wide (8 rows x 58); DR taps write all 464 columns
    (16 garbage), bf16 taps and the evac use tight 8x56 strided views.
  - PSUM evac via VectorE tensor_scalar copy with accum_out row sums; sum of
    squares via ScalarE Square (GpSimd for the last sample to keep ACT/DVE
    free at the tail).
  - Per-sample stats: one fused grouped tensor_reduce + ones-matmul
    partition-broadcast; inv-std via ScalarE Rsqrt (eps dropped: effect
    ~4e-7 relative). The stats matmul of sample b is emitted after sample
    b+1's first conv group so the PE never stalls on the DVE reduce.
  - normalize+relu to fp16: samples 0-2 on ScalarE (hidden under conv);
    the last sample is split ScalarE/VectorE/GpSimd with output DMAs on
    three queues so the serial tail is ~7us instead of ~16us.
  - Output is fp16 (absmax ~5.4, adds <1e-4 error); host upcasts to f32.
"""

import numpy as np
import ml_dtypes
from contextlib import ExitStack

# ---- problem constants (hardcoded per contract) ----
B_FULL, CIN, H, W = 32, 128, 56, 56
COUT, KK = 256, 3
N_CORES = 8
B = B_FULL // N_CORES          # 4 samples per core
HP = H + 2                     # 58 padded rows/cols
SB_B = HP * HP + 4             # per-sample stride in padded x (3368)
YB = 7                         # y-blocks
RPB = H // YB                  # 8 rows per block
NFREE = RPB * HP               # 464 matmul free size (incl 2 garbage cols/row)
NINT = RPB * W                 # 448 interior elements per block
HW = H * W                     # 3136
NELEM = COUT * HW              # 802816 elements per sample for stats
EPS = 1e-5

# conv taps: corners in fp8 DoubleRow pairs, the rest bf16
FP8_PAIRS = [((0, 0), (0, 2)), ((2, 0), (2, 2))]
BF_TAPS = [(0, 1), (1, 0), (1, 1), (1, 2), (2, 1)]

_CACHE = {}
TRACE = False                  # set by test.py to collect an NTFF profile
TRACE_DIR = None
LAST_RESULTS = None


def _off(t):
    return t[0] * HP + t[1]


def _emit(nc, tc, xb_d, x8_d, wb_d, w8_d, bq_d, y_d):
    import concourse.mybir as mybir
    from concourse.ap import AP

    f32 = mybir.dt.float32
    f16 = mybir.dt.float16
    bf16 = mybir.dt.bfloat16
    fp8 = mybir.dt.float8e4
    AF = mybir.ActivationFunctionType
    OP = mybir.AluOpType
    AX = mybir.AxisListType
    DR = mybir.MatmulPerfMode.DoubleRow

    NBF = len(BF_TAPS)

    with ExitStack() as ctx:
        const_pool = ctx.enter_context(tc.tile_pool(name="const", bufs=1))
        xpool = ctx.enter_context(tc.tile_pool(name="x", bufs=1))
        zpool = ctx.enter_context(tc.tile_pool(name="z", bufs=5))
        sqpool = ctx.enter_context(tc.tile_pool(name="sq", bufs=3))
        stpool = ctx.enter_context(tc.tile_pool(name="st", bufs=2))
        outpool = ctx.enter_context(tc.tile_pool(name="out", bufs=8))
        cpsum = ctx.enter_context(tc.tile_pool(name="cps", bufs=7, space="PSUM"))
        spsum = ctx.enter_context(tc.tile_pool(name="sps", bufs=1, space="PSUM"))

        # PE warm-up first in program order: scr memset on GpSimd issues
        # before any DMA work, so the HAM ramp starts while inputs land.
        scr = const_pool.tile([128, 576], bf16)
        nc.gpsimd.memset(scr[:], 0.0)
        for _ in range(10):
            dzt = cpsum.tile([128, NFREE], f32, tag="zt")
            nc.tensor.matmul(dzt[:, 0:448], scr[:, 0:128], scr[:, 0:448],
                             start=True, stop=True)

        wb_sb = const_pool.tile([CIN, 2 * NBF * 128], bf16)
        WHALF = NBF * 128
        nc.sync.dma_start(wb_sb[:, 0:WHALF], wb_d[:, 0:WHALF])
        nc.scalar.dma_start(wb_sb[:, WHALF:], wb_d[:, WHALF:])
        w8_sb = const_pool.tile([CIN, 2 * 2 * 256], fp8)
        nc.gpsimd.dma_start(w8_sb[:], w8_d[:])
        bq_sb = const_pool.tile([128, 5], f32)
        nc.sync.dma_start(bq_sb[:], bq_d[:])
        ones = const_pool.tile([128, 128], f32)
        nc.vector.memset(ones[:], 1.0)

        # x: bf16 copy on the scalar queue, fp8 copy on the vector queue.
        xb_sb = xpool.tile([CIN, B * SB_B], bf16)
        x8_sb = xpool.tile([CIN, B * SB_B], fp8)
        cuts = [0, NFREE + 3 * HP, 3 * NFREE + 2 * HP, 5 * NFREE + 2 * HP, SB_B]
        for i in range(4):
            nc.scalar.dma_start(xb_sb[:, cuts[i]:cuts[i + 1]],
                                xb_d[:, cuts[i]:cuts[i + 1]])
            nc.gpsimd.dma_start(x8_sb[:, cuts[i]:cuts[i + 1]],
                                x8_d[:, cuts[i]:cuts[i + 1]])
        for b in range(1, B):
            s = slice(b * SB_B, (b + 1) * SB_B)
            nc.scalar.dma_start(xb_sb[:, s], xb_d[:, s])
            nc.gpsimd.dma_start(x8_sb[:, s], x8_d[:, s])

        x8_base = x8_sb[:]
        p_stride = list(x8_base.ap[0])

        def dr_rhs(o0, tA, tB):
            return AP(x8_base.tensor, x8_base.offset + o0 + _off(tA),
                      [p_stride, [_off(tB) - _off(tA), 2], [1, NFREE]])

        def emit_stats_and_normalize(b, rsq, z_hb, last):
            # ---- per-sample stats ----
            st6 = stpool.tile([128, 6], f32, tag="st6")
            # fused reduce: [p, 4 groups(stride 8), 7] -> [S0,S1,Q0,Q1]
            r = rsq[:]
            rin = AP(r.tensor, r.offset, [list(r.ap[0]), [8, 4], [1, YB]])
            nc.vector.tensor_reduce(st6[:, 0:4], rin, axis=AX.X, op=OP.add)
            # BR2 = S_c * 2*bq_c
            nc.vector.tensor_tensor(st6[:, 4:6], st6[:, 0:2], bq_sb[:, 3:5],
                                    op=OP.mult)
            st_ps = spsum.tile([128, 6], f32, tag="stps")
            nc.tensor.matmul(st_ps[:], ones[:], st6[:], start=True, stop=True)
            sb_st = stpool.tile([128, 6], f32, tag="sbst")
            nc.vector.tensor_copy(sb_st[:], st_ps[:])

            # scal cols: 0 Stot, 1 mean, 2 SStot, 3 varN, 4 inv, 5 tmp
            scal = stpool.tile([128, 6], f32, tag="scal")
            # S = s0+s1; then Stot = S + C1
            nc.vector.tensor_tensor(scal[:, 0:1], sb_st[:, 0:1], sb_st[:, 1:2],
                                    op=OP.add)
            nc.vector.tensor_tensor(scal[:, 0:1], scal[:, 0:1], bq_sb[:, 2:3],
                                    op=OP.add)
            nc.vector.tensor_scalar_mul(scal[:, 1:2], scal[:, 0:1], 1.0 / NELEM)
            # SStot = (q0+q1) + (br0+br1) + C1
            nc.vector.tensor_tensor(scal[:, 2:3], sb_st[:, 2:3], sb_st[:, 3:4],
                                    op=OP.add)
            nc.vector.tensor_tensor(scal[:, 5:6], sb_st[:, 4:5], sb_st[:, 5:6],
                                    op=OP.add)
            nc.vector.tensor_tensor(scal[:, 2:3], scal[:, 2:3], scal[:, 5:6],
                                    op=OP.add)
            nc.vector.tensor_tensor(scal[:, 2:3], scal[:, 2:3], bq_sb[:, 2:3],
                                    op=OP.add)
            # varN = SStot - Stot^2/NELEM
            nc.vector.scalar_tensor_tensor(scal[:, 5:6], scal[:, 0:1],
                                           1.0 / NELEM, scal[:, 0:1],
                                           op0=OP.mult, op1=OP.mult)
            nc.vector.tensor_tensor(scal[:, 3:4], scal[:, 2:3], scal[:, 5:6],
                                    op=OP.subtract)
            # inv = 1/sqrt(varN/(NELEM-1))  (eps dropped, ~4e-7 effect)
            nc.scalar.activation(scal[:, 4:5], scal[:, 3:4], AF.Sqrt,
                                 scale=1.0 / (NELEM - 1))
            nc.vector.reciprocal(scal[:, 4:5], scal[:, 4:5])
            # b2[h] = (bq[h] - mean) * inv
            b2 = stpool.tile([128, 2], f32, tag="b2")
            for h in range(2):
                nc.vector.scalar_tensor_tensor(b2[:, h:h + 1], bq_sb[:, h:h + 1],
                                               scal[:, 1:2], scal[:, 4:5],
                                               op0=OP.subtract, op1=OP.mult)

            # ---- normalize + relu(fp16) + store ----
            inv = scal[:, 4:5]
            if not last:
                HChunk = HW // 4
                for h in range(2):
                    for ck in range(4):
                        zn = outpool.tile([128, HChunk], f16, tag="zn")
                        zsrc = z_hb[h][:, ck * HChunk:(ck + 1) * HChunk]
                        nc.scalar.activation(zn[:], zsrc, AF.Relu,
                                             bias=b2[:, h:h + 1], scale=inv)
                        nc.sync.dma_start(
                            y_d[b, h * 128:(h + 1) * 128,
                                ck * HChunk:(ck + 1) * HChunk], zn[:])
            else:
                # ScalarE takes 2/3, VectorE 1/3; DMAs on sync + gpsimd queues
                CS = 1024
                CV = HW - 2 * CS      # 1088 VectorE
                for h in range(2):
                    zs = z_hb[h]
                    for ck in range(2):
                        zn_s = outpool.tile([128, CS], f16, tag="zns", bufs=4)
                        nc.scalar.activation(zn_s[:], zs[:, ck * CS:(ck + 1) * CS],
                                             AF.Relu,
                                             bias=b2[:, h:h + 1], scale=inv)
                        nc.sync.dma_start(
                            y_d[b, h * 128:(h + 1) * 128, ck * CS:(ck + 1) * CS],
                            zn_s[:])
                    t_v = outpool.tile([128, CV], f32, tag="tv", bufs=2)
                    nc.vector.tensor_scalar(out=t_v[:], in0=zs[:, 2 * CS:HW],
                                            scalar1=inv, scalar2=b2[:, h:h + 1],
                                            op0=OP.mult, op1=OP.add)
                    zn_v = outpool.tile([128, CV], f16, tag="znv", bufs=2)
                    nc.vector.tensor_scalar_max(zn_v[:], t_v[:], 0.0)
                    nc.gpsimd.dma_start(
                        y_d[b, h * 128:(h + 1) * 128, 2 * CS:HW], zn_v[:])

        pending = None
        for b in range(B):
            z_hb = []
            rsq = stpool.tile([128, 32], f32, tag="rsq")
            for h in range(2):
                z_sb = zpool.tile([128, HW], f32, tag="z")
                z_hb.append(z_sb)
                for yb in range(YB):
                    zt = cpsum.tile([128, NFREE], f32, tag="zt")
                    zt3 = zt[:].rearrange("p (r c) -> p r c", c=HP)[:, :, 0:W]
                    o0 = b * SB_B + yb * NFREE
                    for pi, (tA, tB) in enumerate(FP8_PAIRS):
                        lhs = w8_sb[:, (h * 2 + pi) * 256:(h * 2 + pi + 1) * 256
                                    ].rearrange("p (two m) -> p two m", two=2)
                        nc.tensor.matmul(zt[:], lhs, dr_rhs(o0, tA, tB),
                                         start=(pi == 0), stop=False,
                                         perf_mode=DR)
                    for ti, t in enumerate(BF_TAPS):
                        rhs = xb_sb[:, o0 + _off(t): o0 + _off(t) + NFREE
                                    ].rearrange("p (r c) -> p r c",
                                                c=HP)[:, :, 0:W]
                        nc.tensor.matmul(
                            zt3, wb_sb[:, (h * NBF + ti) * 128:
                                       (h * NBF + ti + 1) * 128], rhs,
                            start=False, stop=(ti == NBF - 1),
                            skip_group_check=True)
                    # evac (strided valid view) + per-channel row sums
                    si = h * 8 + yb
                    nc.vector.tensor_scalar(
                        out=z_sb[:, yb * NINT:(yb + 1) * NINT], in0=zt3,
                        scalar1=1.0, scalar2=0.0,
                        op0=OP.mult, op1=OP.add,
                        accum_out=rsq[:, si:si + 1])
                    zslice = z_sb[:, yb * NINT:(yb + 1) * NINT]
                    sq = sqpool.tile([128, NINT], f32, tag="sq")
                    if b == B - 1:
                        # last sample: sumsq on VectorE so the stats reduce
                        # isn't gated by ACT's FIFO (prev sample's Relus)
                        nc.vector.scalar_tensor_tensor(
                            out=sq[:], in0=zslice, scalar=1.0, in1=zslice,
                            op0=OP.mult, op1=OP.mult,
                            accum_out=rsq[:, 16 + si:17 + si])
                    else:
                        nc.scalar.activation(
                            sq[:], zslice, AF.Square,
                            accum_out=rsq[:, 16 + si:17 + si])
                    # emit the previous sample's stats after this sample's
                    # first conv group so the PE never waits on the DVE chain
                    if pending is not None and h == 0 and yb == 0:
                        emit_stats_and_normalize(*pending, last=False)
                        pending = None
            if b < B - 1:
                pending = (b, rsq, z_hb)
            else:
                emit_stats_and_normalize(b, rsq, z_hb, last=True)


def _build_program():
    import concourse.bacc as bacc
    import concourse.tile as tile
    import concourse.mybir as mybir

    f32 = mybir.dt.float32
    f16 = mybir.dt.float16
    bf16 = mybir.dt.bfloat16
    fp8 = mybir.dt.float8e4

    nc = bacc.Bacc("TRN2", target_bir_lowering=False, debug=False, num_devices=1)

    xb_d = nc.dram_tensor("xb", [CIN, B * SB_B], bf16, kind="ExternalInput").ap()
    x8_d = nc.dram_tensor("x8", [CIN, B * SB_B], fp8, kind="ExternalInput").ap()
    wb_d = nc.dram_tensor("wb", [CIN, 2 * len(BF_TAPS) * 128], bf16,
                          kind="ExternalInput").ap()
    w8_d = nc.dram_tensor("w8", [CIN, 2 * 2 * 256], fp8,
                          kind="ExternalInput").ap()
    bq_d = nc.dram_tensor("bq", [128, 5], f32, kind="ExternalInput").ap()
    y_d = nc.dram_tensor("y", [B, COUT, HW], f16, kind="ExternalOutput").ap()

    with tile.TileContext(nc) as tc:
        _emit(nc, tc, xb_d, x8_d, wb_d, w8_d, bq_d, y_d)

    nc.compile()
    return nc


def _get_program():
    if "nc" not in _CACHE:
        _CACHE["nc"] = _build_program()
    return _CACHE["nc"]


def _binarize(t_np):
    """(t > t.mean()) as f32, matching the reference's jnp computation."""
    try:
        import jax.numpy as jnp
        tj = jnp.asarray(t_np)
        return np.asarray((tj > tj.mean()).astype(jnp.float32))
    except Exception:
        return (t_np > np.float32(t_np.astype(np.float64).mean())).astype(np.float32)


def kernel(x, weight, bias, train_mode=None):
    """Full-input entry point: shards over 8 NeuronCores, returns full output."""
    import time
    last_err = None
    for attempt in range(3):
        try:
            return _kernel_impl(x, weight, bias)
        except Exception as e:  # transient NRT/device errors: back off and retry
            last_err = e
            if attempt < 2:
                time.sleep(20.0 * (attempt + 1))
    raise last_err


def _kernel_impl(x, weight, bias):
    global LAST_RESULTS
    from concourse.bass_utils import run_bass_kernel_spmd

    x = np.asarray(x, dtype=np.float32)
    weight = np.asarray(weight, dtype=np.float32)
    bias = np.asarray(bias, dtype=np.float32)

    wq = _binarize(weight)                       # [256,128,3,3] {0,1}
    bq = _binarize(bias)                         # [256] {0,1}

    w4 = wq.reshape(2, 128, CIN, 3, 3)           # [h, co_l, ci, kh, kw]

    # bf16 taps -> lhsT layout [ci, (h, t, co_l)]
    wb = np.zeros((CIN, 2, len(BF_TAPS), 128), np.float32)
    for ti, (kh, kw) in enumerate(BF_TAPS):
        wb[:, :, ti, :] = w4[:, :, :, kh, kw].transpose(2, 0, 1)
    wb = np.ascontiguousarray(wb.reshape(CIN, -1)).astype(ml_dtypes.bfloat16)

    # fp8 DR pairs -> lhsT layout [ci, (h, pair, two, co_l)]
    w8 = np.zeros((CIN, 2, 2, 2, 128), np.float32)
    for pi, (tA, tB) in enumerate(FP8_PAIRS):
        for two, (kh, kw) in enumerate((tA, tB)):
            w8[:, :, pi, two, :] = w4[:, :, :, kh, kw].transpose(2, 0, 1)
    w8 = np.ascontiguousarray(w8.reshape(CIN, -1)).astype(ml_dtypes.float8_e4m3)

    bq2 = np.zeros((128, 5), np.float32)
    bq2[:, 0] = bq[0:128]
    bq2[:, 1] = bq[128:256]
    bq2[:, 2] = HW * bq.sum()                    # C1 constant, replicated
    bq2[:, 3] = 2.0 * bq[0:128]
    bq2[:, 4] = 2.0 * bq[128:256]

    # x -> padded [b, ci, SB_B] in bf16 and fp8
    xall_b = np.zeros((B_FULL, CIN, SB_B), dtype=ml_dtypes.bfloat16)
    xall_8 = np.zeros((B_FULL, CIN, SB_B), dtype=ml_dtypes.float8_e4m3)
    xv = xall_b[:, :, :HP * HP].reshape(B_FULL, CIN, HP, HP)
    xv[:, :, 1:H + 1, 1:W + 1] = x.astype(ml_dtypes.bfloat16)
    xv8 = xall_8[:, :, :HP * HP].reshape(B_FULL, CIN, HP, HP)
    xv8[:, :, 1:H + 1, 1:W + 1] = x.astype(ml_dtypes.float8_e4m3)

    in_maps = []
    for c in range(N_CORES):
        xc = np.ascontiguousarray(
            xall_b[c * B:(c + 1) * B].transpose(1, 0, 2).reshape(CIN, B * SB_B))
        xc8 = np.ascontiguousarray(
            xall_8[c * B:(c + 1) * B].transpose(1, 0, 2).reshape(CIN, B * SB_B))
        in_maps.append({"xb": xc, "x8": xc8, "wb": wb, "w8": w8, "bq": bq2})

    nc = _get_program()
    kwargs = {}
    if TRACE:
        kwargs = dict(trace=True, tmpdir=TRACE_DIR)
    res = run_bass_kernel_spmd(nc, in_maps, core_ids=list(range(N_CORES)), **kwargs)
    LAST_RESULTS = res

    out = np.concatenate([np.asarray(res.results[c]["y"], dtype=np.float32)
                          for c in range(N_CORES)], axis=0)
    return out.reshape(B_FULL, COUT, H, W)


# revision 17
# speedup vs baseline: 1.0393x; 1.0061x over previous
"""BinaryNormalizedConv2d on 8 Trainium2 NeuronCores.

Reference computation (per full input):
  Wq = (w > mean(w)), bq = (b > mean(b))          # {0,1} f32
  z  = conv2d(x, Wq, stride 1, pad 1) + bq
  z  = (z - mean_b(z)) / (sqrt(var_b(z, ddof=1)) + 1e-5)   # per-sample over (C,H,W)
  out = relu(z)

Sharding: data-parallel over batch (32 -> 4 per core), weights replicated.

Device kernel (per core, B=4, Cin=128, Cout=256, H=W=56):
  - Mixed-precision conv: the 4 corner taps (kh,kw) in {0,2}x{0,2} run as 2
    fp8e4m3 DoubleRow matmuls (2 taps per instruction, ~2 fp8 elem/cycle),
    the remaining 5 taps as bf16 matmuls. Weights {0,1} are exact in both
    dtypes; only x quantization contributes error (measured 1.68e-2
    absmax-relative vs the 2e-2 gate).
  - x stored padded per-sample [Cin=128 partitions, b, 58*58+4] in bf16 AND
    fp8 so each tap (kh,kw) is a flat offset kh*58+kw. DoubleRow rhs is a
    [128, 2, 464] AP whose dim-1 stride pairs two shifted overlapping views.
  - PSUM banks are 464 wide (8 rows x 58); DR taps write all 464 columns
    (16 garbage), bf16 taps and the evac use tight 8x56 strided views.
  - PSUM evac via VectorE tensor_scalar copy with accum_out row sums; sum of
    squares via ScalarE Square (VectorE for the last sample so the stats
    reduce isn't gated by ACT's queued Relus).
  - Per-sample stats: one fused grouped tensor_reduce + ones-matmul
    partition-broadcast; inv-std via ScalarE Sqrt + VectorE reciprocal
    (eps dropped: effect ~4e-7 relative). The stats matmul of sample b is
    emitted after sample b+1's first conv group so the PE never stalls on
    the DVE reduce.
  - normalize+relu to fp16: samples 0-2 on ScalarE (hidden under conv);
    the last sample is split ScalarE/VectorE with output DMAs spread over
    the sync and gpsimd queues so the serial tail shrinks vs baseline.
  - Output is fp16 (absmax ~5.4, adds <1e-4 error); host upcasts to f32.
"""

import numpy as np
import ml_dtypes
from contextlib import ExitStack

# ---- problem constants (hardcoded per contract) ----
B_FULL, CIN, H, W = 32, 128, 56, 56
COUT, KK = 256, 3
N_CORES = 8
B = B_FULL // N_CORES          # 4 samples per core
HP = H + 2                     # 58 padded rows/cols
SB_B = HP * HP + 4             # per-sample stride in padded x (3368)
YB = 7                         # y-blocks
RPB = H // YB                  # 8 rows per block
NFREE = RPB * HP               # 464 matmul free size (incl 2 garbage cols/row)
NINT = RPB * W                 # 448 interior elements per block
HW = H * W                     # 3136
NELEM = COUT * HW              # 802816 elements per sample for stats
EPS = 1e-5

# conv taps: corners in fp8 DoubleRow pairs, the rest bf16
FP8_PAIRS = [((0, 0), (0, 2)), ((2, 0), (2, 2))]
BF_TAPS = [(0, 1), (1, 0), (1, 1), (1, 2), (2, 1)]

_CACHE = {}
TRACE = False                  # set by test.py to collect an NTFF profile
TRACE_DIR = None
LAST_RESULTS = None


def _off(t):
    return t[0] * HP + t[1]


def _emit(nc, tc, xb_d, x8_d, wb_d, w8_d, bq_d, y_d):
    import concourse.mybir as mybir
    from concourse.ap import AP

    f32 = mybir.dt.float32
    f16 = mybir.dt.float16
    bf16 = mybir.dt.bfloat16
    fp8 = mybir.dt.float8e4
    AF = mybir.ActivationFunctionType
    OP = mybir.AluOpType
    AX = mybir.AxisListType
    DR = mybir.MatmulPerfMode.DoubleRow

    NBF = len(BF_TAPS)

    with ExitStack() as ctx:
        const_pool = ctx.enter_context(tc.tile_pool(name="const", bufs=1))
        xpool = ctx.enter_context(tc.tile_pool(name="x", bufs=1))
        zpool = ctx.enter_context(tc.tile_pool(name="z", bufs=5))
        sqpool = ctx.enter_context(tc.tile_pool(name="sq", bufs=3))
        stpool = ctx.enter_context(tc.tile_pool(name="st", bufs=2))
        outpool = ctx.enter_context(tc.tile_pool(name="out", bufs=8))
        cpsum = ctx.enter_context(tc.tile_pool(name="cps", bufs=7, space="PSUM"))
        spsum = ctx.enter_context(tc.tile_pool(name="sps", bufs=1, space="PSUM"))

        # PE warm-up first in program order: scr memset on GpSimd issues
        # before any DMA work, so the HAM ramp starts while inputs land.
        scr = const_pool.tile([128, 576], bf16)
        nc.gpsimd.memset(scr[:], 0.0)
        for _ in range(10):
            dzt = cpsum.tile([128, NFREE], f32, tag="zt")
            nc.tensor.matmul(dzt[:, 0:448], scr[:, 0:128], scr[:, 0:448],
                             start=True, stop=True)

        wb_sb = const_pool.tile([CIN, 2 * NBF * 128], bf16)
        WHALF = NBF * 128
        nc.sync.dma_start(wb_sb[:, 0:WHALF], wb_d[:, 0:WHALF])
        nc.scalar.dma_start(wb_sb[:, WHALF:], wb_d[:, WHALF:])
        w8_sb = const_pool.tile([CIN, 2 * 2 * 256], fp8)
        nc.gpsimd.dma_start(w8_sb[:], w8_d[:])
        bq_sb = const_pool.tile([128, 5], f32)
        nc.sync.dma_start(bq_sb[:], bq_d[:])
        ones = const_pool.tile([128, 128], f32)
        nc.vector.memset(ones[:], 1.0)

        # x: bf16 copy on the scalar queue, fp8 copy on the gpsimd queue.
        xb_sb = xpool.tile([CIN, B * SB_B], bf16)
        x8_sb = xpool.tile([CIN, B * SB_B], fp8)
        cuts = [0, NFREE + 3 * HP, 3 * NFREE + 2 * HP, 5 * NFREE + 2 * HP, SB_B]
        for i in range(4):
            nc.scalar.dma_start(xb_sb[:, cuts[i]:cuts[i + 1]],
                                xb_d[:, cuts[i]:cuts[i + 1]])
            nc.gpsimd.dma_start(x8_sb[:, cuts[i]:cuts[i + 1]],
                                x8_d[:, cuts[i]:cuts[i + 1]])
        for b in range(1, B):
            s = slice(b * SB_B, (b + 1) * SB_B)
            nc.scalar.dma_start(xb_sb[:, s], xb_d[:, s])
            nc.gpsimd.dma_start(x8_sb[:, s], x8_d[:, s])

        x8_base = x8_sb[:]
        p_stride = list(x8_base.ap[0])

        def dr_rhs(o0, tA, tB):
            return AP(x8_base.tensor, x8_base.offset + o0 + _off(tA),
                      [p_stride, [_off(tB) - _off(tA), 2], [1, NFREE]])

        def emit_stats_and_normalize(b, rsq, z_hb, last):
            # ---- per-sample stats ----
            st6 = stpool.tile([128, 6], f32, tag="st6")
            # fused reduce: [p, 4 groups(stride 8), 7] -> [S0,S1,Q0,Q1]
            r = rsq[:]
            rin = AP(r.tensor, r.offset, [list(r.ap[0]), [8, 4], [1, YB]])
            nc.vector.tensor_reduce(st6[:, 0:4], rin, axis=AX.X, op=OP.add)
            # BR2 = S_c * 2*bq_c
            nc.vector.tensor_tensor(st6[:, 4:6], st6[:, 0:2], bq_sb[:, 3:5],
                                    op=OP.mult)
            st_ps = spsum.tile([128, 6], f32, tag="stps")
            nc.tensor.matmul(st_ps[:], ones[:], st6[:], start=True, stop=True)
            sb_st = stpool.tile([128, 6], f32, tag="sbst")
            nc.vector.tensor_copy(sb_st[:], st_ps[:])

            # scal cols: 0 Stot, 1 mean, 2 SStot, 3 varN, 4 inv, 5 tmp
            scal = stpool.tile([128, 6], f32, tag="scal")
            # S = s0+s1; then Stot = S + C1
            nc.vector.tensor_tensor(scal[:, 0:1], sb_st[:, 0:1], sb_st[:, 1:2],
                                    op=OP.add)
            nc.vector.tensor_tensor(scal[:, 0:1], scal[:, 0:1], bq_sb[:, 2:3],
                                    op=OP.add)
            nc.vector.tensor_scalar_mul(scal[:, 1:2], scal[:, 0:1], 1.0 / NELEM)
            # SStot = (q0+q1) + (br0+br1) + C1
            nc.vector.tensor_tensor(scal[:, 2:3], sb_st[:, 2:3], sb_st[:, 3:4],
                                    op=OP.add)
            nc.vector.tensor_tensor(scal[:, 5:6], sb_st[:, 4:5], sb_st[:, 5:6],
                                    op=OP.add)
            nc.vector.tensor_tensor(scal[:, 2:3], scal[:, 2:3], scal[:, 5:6],
                                    op=OP.add)
            nc.vector.tensor_tensor(scal[:, 2:3], scal[:, 2:3], bq_sb[:, 2:3],
                                    op=OP.add)
            # varN = SStot - Stot^2/NELEM
            nc.vector.scalar_tensor_tensor(scal[:, 5:6], scal[:, 0:1],
                                           1.0 / NELEM, scal[:, 0:1],
                                           op0=OP.mult, op1=OP.mult)
            nc.vector.tensor_tensor(scal[:, 3:4], scal[:, 2:3], scal[:, 5:6],
                                    op=OP.subtract)
            # inv = 1/sqrt(varN/(NELEM-1))  (eps dropped, ~4e-7 effect)
            nc.scalar.activation(scal[:, 4:5], scal[:, 3:4], AF.Sqrt,
                                 scale=1.0 / (NELEM - 1))
            nc.vector.reciprocal(scal[:, 4:5], scal[:, 4:5])
            # b2[h] = (bq[h] - mean) * inv
            b2 = stpool.tile([128, 2], f32, tag="b2")
            for h in range(2):
                nc.vector.scalar_tensor_tensor(b2[:, h:h + 1], bq_sb[:, h:h + 1],
                                               scal[:, 1:2], scal[:, 4:5],
                                               op0=OP.subtract, op1=OP.mult)

            # ---- normalize + relu(fp16) + store ----
            inv = scal[:, 4:5]
            if not last:
                HChunk = HW // 4
                for h in range(2):
                    for ck in range(4):
                        zn = outpool.tile([128, HChunk], f16, tag="zn")
                        zsrc = z_hb[h][:, ck * HChunk:(ck + 1) * HChunk]
                        nc.scalar.activation(zn[:], zsrc, AF.Relu,
                                             bias=b2[:, h:h + 1], scale=inv)
                        nc.sync.dma_start(
                            y_d[b, h * 128:(h + 1) * 128,
                                ck * HChunk:(ck + 1) * HChunk], zn[:])
            else:
                # one big ScalarE chunk + one DVE chunk per half; each chunk's
                # DMA is split in two, balanced across sync/gpsimd queues
                CS = 1708
                CV = HW - CS          # 1428 VectorE
                CSH, CVH = CS // 2, CV // 2
                for h in range(2):
                    zs = z_hb[h]
                    row = y_d[b, h * 128:(h + 1) * 128]
                    zn_s = outpool.tile([128, CS], f16, tag="zns", bufs=2)
                    nc.scalar.activation(zn_s[:], zs[:, 0:CS], AF.Relu,
                                         bias=b2[:, h:h + 1], scale=inv)
                    nc.sync.dma_start(row[:, 0:CSH], zn_s[:, 0:CSH])
                    nc.gpsimd.dma_start(row[:, CSH:CS], zn_s[:, CSH:CS])
                    t_v = outpool.tile([128, CV], f32, tag="tv", bufs=2)
                    nc.vector.tensor_scalar(out=t_v[:], in0=zs[:, CS:HW],
                                            scalar1=inv, scalar2=b2[:, h:h + 1],
                                            op0=OP.mult, op1=OP.add)
                    zn_v = outpool.tile([128, CV], f16, tag="znv", bufs=2)
                    nc.vector.tensor_scalar_max(zn_v[:], t_v[:], 0.0)
                    qa, qb = (nc.gpsimd, nc.sync) if h == 0 else (nc.sync, nc.gpsimd)
                    qa.dma_start(row[:, CS:CS + CVH], zn_v[:, 0:CVH])
                    qb.dma_start(row[:, CS + CVH:HW], zn_v[:, CVH:CV])

        pending = None
        for b in range(B):
            z_hb = []
            rsq = stpool.tile([128, 32], f32, tag="rsq")
            for h in range(2):
                z_sb = zpool.tile([128, HW], f32, tag="z")
                z_hb.append(z_sb)
                for yb in range(YB):
                    zt = cpsum.tile([128, NFREE], f32, tag="zt")
                    zt3 = zt[:].rearrange("p (r c) -> p r c", c=HP)[:, :, 0:W]
                    o0 = b * SB_B + yb * NFREE
                    for pi, (tA, tB) in enumerate(FP8_PAIRS):
                        lhs = w8_sb[:, (h * 2 + pi) * 256:(h * 2 + pi + 1) * 256
                                    ].rearrange("p (two m) -> p two m", two=2)
                        nc.tensor.matmul(zt[:], lhs, dr_rhs(o0, tA, tB),
                                         start=(pi == 0), stop=False,
                                         perf_mode=DR)
                    for ti, t in enumerate(BF_TAPS):
                        rhs = xb_sb[:, o0 + _off(t): o0 + _off(t) + NFREE
                                    ].rearrange("p (r c) -> p r c",
                                                c=HP)[:, :, 0:W]
                        nc.tensor.matmul(
                            zt3, wb_sb[:, (h * NBF + ti) * 128:
                                       (h * NBF + ti + 1) * 128], rhs,
                            start=False, stop=(ti == NBF - 1),
                            skip_group_check=True)
                    # evac (strided valid view) + per-channel row sums
                    si = h * 8 + yb
                    nc.vector.tensor_scalar(
                        out=z_sb[:, yb * NINT:(yb + 1) * NINT], in0=zt3,
                        scalar1=1.0, scalar2=0.0,
                        op0=OP.mult, op1=OP.add,
                        accum_out=rsq[:, si:si + 1])
                    zslice = z_sb[:, yb * NINT:(yb + 1) * NINT]
                    sq = sqpool.tile([128, NINT], f32, tag="sq")
                    if b == B - 1:
                        # last sample: sumsq on VectorE so the stats reduce
                        # isn't gated by ACT's FIFO (prev sample's Relus)
                        nc.vector.scalar_tensor_tensor(
                            out=sq[:], in0=zslice, scalar=1.0, in1=zslice,
                            op0=OP.mult, op1=OP.mult,
                            accum_out=rsq[:, 16 + si:17 + si])
                    else:
                        nc.scalar.activation(
                            sq[:], zslice, AF.Square,
                            accum_out=rsq[:, 16 + si:17 + si])
                    # emit the previous sample's stats after this sample's
                    # first conv group so the PE never waits on the DVE chain
                    if pending is not None and h == 0 and yb == 0:
                        emit_stats_and_normalize(*pending, last=False)
                        pending = None
            if b < B - 1:
                pending = (b, rsq, z_hb)
            else:
                emit_stats_and_normalize(b, rsq, z_hb, last=True)


def _build_program():
    import concourse.bacc as bacc
    import concourse.tile as tile
    import concourse.mybir as mybir

    f32 = mybir.dt.float32
    f16 = mybir.dt.float16
    bf16 = mybir.dt.bfloat16
    fp8 = mybir.dt.float8e4

    nc = bacc.Bacc("TRN2", target_bir_lowering=False, debug=False, num_devices=1)

    xb_d = nc.dram_tensor("xb", [CIN, B * SB_B], bf16, kind="ExternalInput").ap()
    x8_d = nc.dram_tensor("x8", [CIN, B * SB_B], fp8, kind="ExternalInput").ap()
    wb_d = nc.dram_tensor("wb", [CIN, 2 * len(BF_TAPS) * 128], bf16,
                          kind="ExternalInput").ap()
    w8_d = nc.dram_tensor("w8", [CIN, 2 * 2 * 256], fp8,
                          kind="ExternalInput").ap()
    bq_d = nc.dram_tensor("bq", [128, 5], f32, kind="ExternalInput").ap()
    y_d = nc.dram_tensor("y", [B, COUT, HW], f16, kind="ExternalOutput").ap()

    with tile.TileContext(nc) as tc:
        _emit(nc, tc, xb_d, x8_d, wb_d, w8_d, bq_d, y_d)

    nc.compile()
    return nc


def _get_program():
    if "nc" not in _CACHE:
        _CACHE["nc"] = _build_program()
    return _CACHE["nc"]


def _binarize(t_np):
    """(t > t.mean()) as f32, matching the reference's jnp computation."""
    try:
        import jax.numpy as jnp
        tj = jnp.asarray(t_np)
        return np.asarray((tj > tj.mean()).astype(jnp.float32))
    except Exception:
        return (t_np > np.float32(t_np.astype(np.float64).mean())).astype(np.float32)


def kernel(x, weight, bias, train_mode=None):
    """Full-input entry point: shards over 8 NeuronCores, returns full output."""
    import time
    last_err = None
    for attempt in range(3):
        try:
            return _kernel_impl(x, weight, bias)
        except Exception as e:  # transient NRT/device errors: back off and retry
            last_err = e
            if attempt < 2:
                time.sleep(20.0 * (attempt + 1))
    raise last_err


def _kernel_impl(x, weight, bias):
    global LAST_RESULTS
    from concourse.bass_utils import run_bass_kernel_spmd

    x = np.asarray(x, dtype=np.float32)
    weight = np.asarray(weight, dtype=np.float32)
    bias = np.asarray(bias, dtype=np.float32)

    wq = _binarize(weight)                       # [256,128,3,3] {0,1}
    bq = _binarize(bias)                         # [256] {0,1}

    w4 = wq.reshape(2, 128, CIN, 3, 3)           # [h, co_l, ci, kh, kw]

    # bf16 taps -> lhsT layout [ci, (h, t, co_l)]
    wb = np.zeros((CIN, 2, len(BF_TAPS), 128), np.float32)
    for ti, (kh, kw) in enumerate(BF_TAPS):
        wb[:, :, ti, :] = w4[:, :, :, kh, kw].transpose(2, 0, 1)
    wb = np.ascontiguousarray(wb.reshape(CIN, -1)).astype(ml_dtypes.bfloat16)

    # fp8 DR pairs -> lhsT layout [ci, (h, pair, two, co_l)]
    w8 = np.zeros((CIN, 2, 2, 2, 128), np.float32)
    for pi, (tA, tB) in enumerate(FP8_PAIRS):
        for two, (kh, kw) in enumerate((tA, tB)):
            w8[:, :, pi, two, :] = w4[:, :, :, kh, kw].transpose(2, 0, 1)
    w8 = np.ascontiguousarray(w8.reshape(CIN, -1)).astype(ml_dtypes.float8_e4m3)

    bq2 = np.zeros((128, 5), np.float32)
    bq2[:, 0] = bq[0:128]
    bq2[:, 1] = bq[128:256]
    bq2[:, 2] = HW * bq.sum()                    # C1 constant, replicated
    bq2[:, 3] = 2.0 * bq[0:128]
    bq2[:, 4] = 2.0 * bq[128:256]

    # x -> padded [b, ci, SB_B] in bf16 and fp8
    xall_b = np.zeros((B_FULL, CIN, SB_B), dtype=ml_dtypes.bfloat16)
    xall_8 = np.zeros((B_FULL, CIN, SB_B), dtype=ml_dtypes.float8_e4m3)
    xv = xall_b[:, :, :HP * HP].reshape(B_FULL, CIN, HP, HP)
    xv[:, :, 1:H + 1, 1:W + 1] = x.astype(ml_dtypes.bfloat16)
    xv8 = xall_8[:, :, :HP * HP].reshape(B_FULL, CIN, HP, HP)
    xv8[:, :, 1:H + 1, 1:W + 1] = x.astype(ml_dtypes.float8_e4m3)

    in_maps = []
    for c in range(N_CORES):
        xc = np.ascontiguousarray(
            xall_b[c * B:(c + 1) * B].transpose(1, 0, 2).reshape(CIN, B * SB_B))
        xc8 = np.ascontiguousarray(
            xall_8[c * B:(c + 1) * B].transpose(1, 0, 2).reshape(CIN, B * SB_B))
        in_maps.append({"xb": xc, "x8": xc8, "wb": wb, "w8": w8, "bq": bq2})

    nc = _get_program()
    kwargs = {}
    if TRACE:
        kwargs = dict(trace=True, tmpdir=TRACE_DIR)
    res = run_bass_kernel_spmd(nc, in_maps, core_ids=list(range(N_CORES)), **kwargs)
    LAST_RESULTS = res

    out = np.concatenate([np.asarray(res.results[c]["y"], dtype=np.float32)
                          for c in range(N_CORES)], axis=0)
    return out.reshape(B_FULL, COUT, H, W)
